# revision 1
# baseline (speedup 1.0000x reference)
"""Per-core causal self-attention kernel builder (Bass/Tile, TRN2).

Implements, for one core's shard (one batch b, one head-group of HL heads):
    K  = x @ Wk + bk                    # [T, NW], NW = HL*64
    per head h: S = K_h K_h^T / 8 (causal), P = softmax rows (no max-sub)
    Y_h = P @ K_h                       # q = k = v quirk of the reference
    out_partial = Y @ Wp                # [T, COUT]; host sums partials + b_proj

Layout strategy: everything is computed transposed.
  KT[n, t] (n = h*64+d) via matmul(lhsT=wk, rhs=xT);
  S^T strips [j-block, i-chunk] via matmul(lhsT=KT jb-block, rhs=KT i-slice),
  two heads run concurrently on PE row-groups 0-63 / 64-127, their strips
  land in one 2-bank PSUM tile so a single ACT op exps both;
  E^T = exp(S^T/8) on ACT straight out of PSUM, causal diag masked by a
  0/1 upper-tri tile on DVE;
  U^T accumulated per i-chunk via matmul(lhsT=V_aug, rhs=E^T) where V_aug is
  K natural layout (from PE transpose of KT) with ones in the other half of
  the 128-col slot -> rows carry U^T and the softmax denominator replicated;
  normalize with DVE reciprocal_approx + small SBUF->SBUF DMA partition shift;
  proj via matmul(lhsT=Y^T block, rhs=wp), interleaved into the last
  head-pair's chunk loop.

Pipeline: phases are emitted interleaved per head-pair (KT, V, ATT) so the
tile scheduler overlaps the ACT-bound attention with the next pair's PE work.

Dtypes: dt_s is the score path (xT, wk, KT, ident), dt_v the value path
(V, E, Y, Wp, tri). float32r = fp32 storage, PE rounds; full PE rate at N>=256.
"""

from contextlib import ExitStack

import concourse.bass as bass
import concourse.tile as tile
from concourse import mybir

F32 = mybir.dt.float32
F32R = mybir.dt.float32r
BF16 = mybir.dt.bfloat16
FP16 = mybir.dt.float16
EXP = mybir.ActivationFunctionType.Exp


class Cfg:
    def __init__(self, T=2048, CIN=1024, HL=8, COUT=1024, dt_s=F32R, dt_v=BF16,
                 dt_x=BF16):
        self.T, self.CIN, self.HL, self.COUT = T, CIN, HL, COUT
        self.dt_s, self.dt_v, self.dt_x = dt_s, dt_v, dt_x
        assert HL % 2 == 0 and T % 512 == 0 and CIN % 128 == 0 and COUT % 512 == 0
        self.NW = HL * 64          # n width (local head dims)
        self.NB = self.NW // 128   # head-pair blocks
        self.TB = T // 128         # t blocks
        self.NCH = T // 512        # i chunks
        self.CB = CIN // 128       # c chunks


def np_dt(dt):
    import numpy as np
    import ml_dtypes
    if dt == BF16:
        return ml_dtypes.bfloat16
    if dt == FP16:
        return np.float16
    return np.float32


def declare_io(nc, cfg):
    io = {}
    io["xT"] = nc.dram_tensor("xT", [cfg.CIN, cfg.T], cfg.dt_x, kind="ExternalInput")
    io["wk"] = nc.dram_tensor("wk", [cfg.CIN, cfg.NW], cfg.dt_x, kind="ExternalInput")
    io["bk"] = nc.dram_tensor("bk", [cfg.NW, 1], F32, kind="ExternalInput")
    io["wp"] = nc.dram_tensor("wp", [cfg.NW, cfg.COUT], cfg.dt_v, kind="ExternalInput")
    io["tri"] = nc.dram_tensor("tri", [128, 128], cfg.dt_v, kind="ExternalInput")
    io["ident"] = nc.dram_tensor("ident", [128, 128], cfg.dt_s, kind="ExternalInput")
    io["out"] = nc.dram_tensor("out", [cfg.T, cfg.COUT], F32, kind="ExternalOutput")
    return io


def build(ctx: ExitStack, tc: tile.TileContext, io, cfg: Cfg):
    nc = tc.nc
    T, HL, NB, TB, NCH, CB, COUT = cfg.T, cfg.HL, cfg.NB, cfg.TB, cfg.NCH, cfg.CB, cfg.COUT
    dt_s, dt_v, dt_x = cfg.dt_s, cfg.dt_v, cfg.dt_x

    consts = ctx.enter_context(tc.tile_pool(name="consts", bufs=1))
    # PSUM budget (8 banks): s 2x[128,1024]=4, u 2x[128,512]=2, ktv 1x[128,1024]=2
    spsum = ctx.enter_context(tc.tile_pool(name="sps", bufs=2, space="PSUM"))
    upsum = ctx.enter_context(tc.tile_pool(name="ups", bufs=1, space="PSUM"))
    kpsum = ctx.enter_context(tc.tile_pool(name="kps", bufs=2, space="PSUM"))
    upool = ctx.enter_context(tc.tile_pool(name="usb", bufs=2))
    epool = ctx.enter_context(tc.tile_pool(name="e", bufs=8))
    rpool = ctx.enter_context(tc.tile_pool(name="r", bufs=4))
    opool = ctx.enter_context(tc.tile_pool(name="o", bufs=3))

    # ---- persistent SBUF tensors ----
    tri_t = consts.tile([128, 128], dt_v, tag="tri")
    nc.sync.dma_start(tri_t[:], io["tri"].ap())
    id_t = consts.tile([128, 128], dt_s, tag="ident")
    nc.sync.dma_start(id_t[:], io["ident"].ap())

    xT_t, wk_t, kt_t, bk_t, wp_t, v_t, y_t = [], [], [], [], [], [], []
    for cc in range(CB):
        t = consts.tile([128, cfg.NW], dt_x, tag=f"wk{cc}", name=f"wk{cc}")
        nc.sync.dma_start(t[:], io["wk"].ap()[cc * 128:(cc + 1) * 128, :])
        wk_t.append(t)
    for cc in range(CB):
        xT_t.append(consts.tile([128, T], dt_x, tag=f"xT{cc}", name=f"xT{cc}"))
    for lo, hi in ((0, 512), (512, T)):
        for cc in range(CB):
            nc.sync.dma_start(
                xT_t[cc][:, lo:hi],
                io["xT"].ap()[cc * 128:(cc + 1) * 128, lo:hi])
    for nb in range(NB):
        t = consts.tile([128, 1], F32, tag=f"bk{nb}", name=f"bk{nb}")
        nc.sync.dma_start(t[:], io["bk"].ap()[nb * 128:(nb + 1) * 128, :])
        bk_t.append(t)
        kt_t.append(consts.tile([128, T], dt_s, tag=f"kt{nb}", name=f"kt{nb}"))
        y_t.append(consts.tile([128, T], dt_v, tag=f"y{nb}", name=f"y{nb}"))
    for nb in range(NB):
        t = consts.tile([128, COUT], dt_v, tag=f"wp{nb}", name=f"wp{nb}")
        nc.sync.dma_start(t[:], io["wp"].ap()[nb * 128:(nb + 1) * 128, :])
        wp_t.append(t)
    for tb in range(TB):
        # ones everywhere (idle GpSimd); K quarters overwritten by the V phase
        t = consts.tile([128, HL * 128], dt_v, tag=f"v{tb}", name=f"v{tb}")
        nc.gpsimd.memset(t[:], 1.0)
        v_t.append(t)

    def kt_phase(nb):
        # KT[n,t] = (x @ Wk + bk)^T for this head pair, biased, cast to dt_s
        for tch in range(NCH):
            ps = kpsum.tile([128, 512], F32, tag="kps", name="pskt")
            for cc in range(CB):
                nc.tensor.matmul(
                    ps[:],
                    wk_t[cc][:, nb * 128:(nb + 1) * 128],
                    xT_t[cc][:, tch * 512:(tch + 1) * 512],
                    start=(cc == 0), stop=(cc == CB - 1),
                )
            nc.vector.tensor_scalar_add(
                kt_t[nb][:, tch * 512:(tch + 1) * 512], ps[:], bk_t[nb][:])

    def v_phase(nb):
        # V_aug slots from PE-transposed KT. Slot layout per pair nb
        # (cols nb*256 .. nb*256+256): [K_even(64) | ones | ones | K_odd(64)]
        for tb in range(TB):
            ps = kpsum.tile([128, 512], dt_s, tag="kps", name="pst")
            nc.tensor.transpose(ps[:, 0:128], kt_t[nb][:, tb * 128:(tb + 1) * 128],
                                id_t[:])
            base = nb * 256
            nc.vector.tensor_copy(v_t[tb][:, base:base + 64], ps[:, 0:64])
            nc.vector.tensor_copy(v_t[tb][:, base + 192:base + 256], ps[:, 64:128])

    def proj_phase(tb):
        ot = opool.tile([128, COUT], F32, tag="o", name="ot")
        for nh in range(COUT // 512):
            po = kpsum.tile([128, 512], F32, tag="kps", name="po")
            for hp2 in range(NB):
                nc.tensor.matmul(po[:],
                                 y_t[hp2][:, tb * 128:(tb + 1) * 128],
                                 wp_t[hp2][:, nh * 512:(nh + 1) * 512],
                                 start=(hp2 == 0), stop=(hp2 == NB - 1))
            nc.vector.tensor_copy(ot[:, nh * 512:(nh + 1) * 512], po[:])
        nc.sync.dma_start(io["out"].ap()[tb * 128:(tb + 1) * 128, :], ot[:])

    def att_phase(hp, with_proj):
        cis = list(range(NCH))
        if with_proj:
            cis = cis[::-1]  # big chunk first -> small final chunk, shorter tail
        for ci in cis:
            uAB = upsum.tile([128, 1024], F32, tag="u", name="uAB")
            uA, uB = uAB[:, 0:512], uAB[:, 512:1024]
            jmax = (ci + 1) * 4
            for jb in range(jmax):
                off = max(0, 128 * jb - 512 * ci)
                N = 512 - off
                ilo = 512 * ci + off
                ps = spsum.tile([128, 1024], F32, tag="sps", name="psS")
                nc.tensor.matmul(ps[:, 0:N],
                                 kt_t[hp][0:64, jb * 128:(jb + 1) * 128],
                                 kt_t[hp][0:64, ilo:ilo + N],
                                 start=True, stop=True)
                nc.tensor.matmul(ps[:, 512:512 + N],
                                 kt_t[hp][64:128, jb * 128:(jb + 1) * 128],
                                 kt_t[hp][64:128, ilo:ilo + N],
                                 start=True, stop=True)
                et = epool.tile([128, 1024], dt_v, tag="e", name="et")
                nc.scalar.activation(
                    et[:].rearrange("p (a c) -> p a c", a=2)[:, :, 0:N],
                    ps[:].rearrange("p (a c) -> p a c", a=2)[:, :, 0:N],
                    EXP, scale=0.125)
                if jb >= 4 * ci:  # strip starts at the diagonal block
                    nc.vector.tensor_mul(et[:, 0:128], et[:, 0:128], tri_t[:])
                    nc.vector.tensor_mul(et[:, 512:640], et[:, 512:640], tri_t[:])
                st, sp = (jb == 0), (jb == jmax - 1)
                nc.tensor.matmul(uAB[:, off:off + N],
                                 v_t[jb][:, hp * 256:hp * 256 + 128],
                                 et[:, 0:N], start=st, stop=sp)
                nc.tensor.matmul(uAB[:, 512 + off:512 + off + N],
                                 v_t[jb][:, hp * 256 + 128:hp * 256 + 256],
                                 et[:, 512:512 + N], start=st, stop=sp)
            # normalize: uA rows 0-63 = U^T_even, rows 64-127 = l_even (rep).
            #            uB rows 0-63 = l_odd (rep), rows 64-127 = U^T_odd.
            # Copy U PSUM -> SBUF first so the PSUM slots free up fast.
            us = upool.tile([128, 1024], F32, tag="us", name="us")
            nc.vector.tensor_copy(us[:], uAB[:])
            rA = rpool.tile([128, 512], F32, tag="r", name="rA")
            rB = rpool.tile([128, 512], F32, tag="r", name="rB")
            cs = slice(ci * 512, (ci + 1) * 512)
            nc.vector.reciprocal(rA[64:128, :], us[64:128, 0:512])
            nc.sync.dma_start(rA[0:64, :], rA[64:128, :])
            nc.vector.tensor_mul(y_t[hp][0:64, cs], us[0:64, 0:512], rA[0:64, :])
            nc.vector.reciprocal(rB[0:64, :], us[0:64, 512:1024])
            nc.sync.dma_start(rB[64:128, :], rB[0:64, :])
            nc.vector.tensor_mul(y_t[hp][64:128, cs], us[64:128, 512:1024], rB[64:128, :])
            if with_proj:
                for tb in range(ci * 4, (ci + 1) * 4):
                    proj_phase(tb)

    for hp in range(NB):
        kt_phase(hp)
        v_phase(hp)
        att_phase(hp, with_proj=(hp == NB - 1))


def make_inputs(cfg, x, Wk, bk, Wp):
    """Host-side input map for one core. x [T,CIN] fp32, Wk [CIN,NW], bk [NW], Wp [NW,COUT]."""
    import numpy as np
    nds, ndv, ndx = np_dt(cfg.dt_s), np_dt(cfg.dt_v), np_dt(cfg.dt_x)
    jj, ii = np.meshgrid(np.arange(128), np.arange(128), indexing="ij")
    m = {
        "xT": np.ascontiguousarray(x.T).astype(ndx),
        "wk": np.ascontiguousarray(Wk).astype(ndx),
        "bk": np.ascontiguousarray(bk.reshape(-1, 1)).astype(np.float32),
        "wp": np.ascontiguousarray(Wp).astype(ndv),
        "tri": (jj <= ii).astype(ndv),
        "ident": np.eye(128).astype(nds),
    }
    return m


def ref_core(x, Wk, bk, Wp, HL):
    """Numpy replica of what one core computes (without b_proj)."""
    import numpy as np
    T, CIN = x.shape
    K = x.astype(np.float64) @ Wk.astype(np.float64) + bk.astype(np.float64)
    out = np.zeros((T, Wp.shape[1]), dtype=np.float64)
    for h in range(HL):
        Kh = K[:, h * 64:(h + 1) * 64]
        S = Kh @ Kh.T / 8.0
        mask = np.tril(np.ones((T, T), dtype=bool))
        S = np.where(mask, S, -np.inf)
        S = S - S.max(axis=1, keepdims=True)
        P = np.exp(S)
        P /= P.sum(axis=1, keepdims=True)
        Y = P @ Kh
        out += Y @ Wp[h * 64:(h + 1) * 64, :].astype(np.float64)
    return out


# ======================================================================
# Host-side entry: shard across 8 NeuronCores as (batch x head-group),
# run the Bass kernel, gather + reduce partials on host.
# ======================================================================

import numpy as np

from concourse import bacc
from concourse.bass_utils import run_bass_kernel_spmd

B, T, C, H = 4, 2048, 1024, 16
N_CORES = 8
HG = 2                      # head groups (tensor-parallel axis)
NW = C // HG                # 512 columns of W_k per group

_cache = {}


def get_compiled(dt_s=FP16, dt_v=FP16):
    key = (dt_s, dt_v)
    if key not in _cache:
        cfg = Cfg(T=T, CIN=C, HL=H // HG, COUT=C, dt_s=dt_s, dt_v=dt_v)
        nc = bacc.Bacc("TRN2", target_bir_lowering=False, debug=False,
                       num_devices=N_CORES)
        io = declare_io(nc, cfg)
        with tile.TileContext(nc) as tc:
            with ExitStack() as ctx:
                build(ctx, tc, io, cfg)
        nc.compile()
        _cache[key] = (nc, cfg)
    return _cache[key]


def make_in_maps(cfg, x, W_attn, b_attn, W_proj):
    in_maps = []
    for core in range(N_CORES):
        b, hg = core // HG, core % HG
        sl = slice(C + hg * NW, C + (hg + 1) * NW)
        in_maps.append(make_inputs(
            cfg, x[b], W_attn[:, sl], b_attn[sl],
            W_proj[hg * NW:(hg + 1) * NW, :]))
    return in_maps


def kernel(x, W_attn, b_attn, W_proj, b_proj):
    x = np.asarray(x, dtype=np.float32)
    W_attn = np.asarray(W_attn, dtype=np.float32)
    b_attn = np.asarray(b_attn, dtype=np.float32)
    W_proj = np.asarray(W_proj, dtype=np.float32)
    b_proj = np.asarray(b_proj, dtype=np.float32)

    nc, cfg = get_compiled()
    in_maps = make_in_maps(cfg, x, W_attn, b_attn, W_proj)
    res = run_bass_kernel_spmd(nc, in_maps, core_ids=list(range(N_CORES)))
    out = np.empty((B, T, C), dtype=np.float32)
    for b in range(B):
        out[b] = res.results[HG * b]["out"] + res.results[HG * b + 1]["out"] \
            + b_proj[None, :]
    return out

